# revision 1
# baseline (speedup 1.0000x reference)
"""Trainium2 Bass kernel for masked multi-head attention (B=2,H=16,S=2048,D=64).

Returns (out, p_attn) like the reference:
    scores = q @ k.T / sqrt(D); masked with -1e9 where mask==0
    p_attn = softmax(scores); out = p_attn @ v

Sharding: batch x head across 8 cores. Core c handles batch b=c//4 and heads
4*(c%4)..4*(c%4)+3; no cross-core communication.

Per-core pipeline (all shapes per head unless noted):
  - mask (shared by the core's 4 heads) is converted once to fp8 {-240, 0}
    additive form; it is folded into the scores PSUM via an identity-weight
    matmul (I @ maskneg accumulates elementwise into the scores bank).
  - scores: S tile = qT_chunk.T @ kT (f32r matmuls, N=512) on top of maskneg.
  - ScalarE computes exp(0.125 * psum) emitting row-sum partials (accum_out).
  - P@V path: exp tiles cast to bf16, transposed on TensorE (via identity),
    then oT = v_chunk.T @ eT accumulated over kv chunks; oT is un-transposed
    and scaled by the row reciprocals at the end (the 1/rowsum factors out of
    the kv contraction).
  - p_attn = e * recip (per-partition scalar), streamed to HBM in fp32.
"""

import sys

sys.path.insert(0, "/opt/trn_rl_repo")

import numpy as np

import concourse.bass as bass  # noqa: F401  (engine types via nc)
import concourse.mybir as mybir
import concourse.tile as tile
from concourse import bacc
from concourse.bass_utils import run_bass_kernel_spmd
from concourse.masks import make_identity

F32 = mybir.dt.float32
F32R = mybir.dt.float32r
BF16 = mybir.dt.bfloat16
FP8 = mybir.dt.float8e4  # IEEE e4m3: max finite 240
I32 = mybir.dt.int32

MASK_NEG = 240.0  # additive mask constant; exp(-240/8) ~ 1e-13 ~ 0
AF = mybir.ActivationFunctionType


def build_nc(HP=4, S=2048, D=64, n_cores=8):
    """Build the per-core SPMD program. HP = heads per core."""
    QT = 128                      # q rows per tile
    NB = 512                      # matmul moving-dim block
    QG = min(512, S)              # q rows per group (transpose/PV granularity)
    KVB = min(1024, S)            # kv cols per exp instruction (2 psum banks)
    n_qt = QG // QT               # q tiles per group
    n_qg = S // QG
    n_kvb = S // KVB
    n_nb_kvb = KVB // NB          # matmul blocks per exp block
    n_kc = S // 128               # kv chunks (transpose/PV granularity)
    assert S % KVB == 0 and KVB % NB == 0 and S % 128 == 0

    nc = bacc.Bacc("TRN2", target_bir_lowering=False, debug=False)
    qT_d = nc.dram_tensor("qT", [HP, D, S], F32, kind="ExternalInput").ap()
    kT_d = nc.dram_tensor("kT", [HP, D, S], F32, kind="ExternalInput").ap()
    v_d = nc.dram_tensor("v", [HP, S, D], F32, kind="ExternalInput").ap()
    mask_d = nc.dram_tensor("mask", [S, S], I32, kind="ExternalInput").ap()
    o_d = nc.dram_tensor("o", [HP, S, D], F32, kind="ExternalOutput").ap()
    p_d = nc.dram_tensor("p", [HP, S, S], F32, kind="ExternalOutput").ap()

    with tile.TileContext(nc) as tc:
        _emit(nc, tc, qT_d, kT_d, v_d, mask_d, o_d, p_d,
              HP=HP, S=S, D=D, QT=QT, NB=NB, n_qt=n_qt, n_qg=n_qg,
              n_kvb=n_kvb, n_nb_kvb=n_nb_kvb, n_kc=n_kc, KVB=KVB, QG=QG)
    nc.compile()
    return nc


def _emit(nc, tc, qT_d, kT_d, v_d, mask_d, o_d, p_d, *,
          HP, S, D, QT, NB, n_qt, n_qg, n_kvb, n_nb_kvb, n_kc, KVB, QG):
    import contextlib

    with contextlib.ExitStack() as ctx:
        singles = ctx.enter_context(tc.tile_pool(name="singles", bufs=1))
        mprep = ctx.enter_context(tc.tile_pool(name="mprep", bufs=2))
        loadp = ctx.enter_context(tc.tile_pool(name="loadp", bufs=2))
        roundp = ctx.enter_context(tc.tile_pool(name="roundp", bufs=2))
        ep = ctx.enter_context(tc.tile_pool(name="ep", bufs=n_qt + 2))
        ebfp = ctx.enter_context(tc.tile_pool(name="ebfp", bufs=n_qt + 1))
        etsp = ctx.enter_context(tc.tile_pool(name="etsp", bufs=3))
        smallp = ctx.enter_context(tc.tile_pool(name="smallp", bufs=2))
        statp = ctx.enter_context(tc.tile_pool(name="statp", bufs=2))
        psp = ctx.enter_context(tc.tile_pool(name="psp", bufs=2, space="PSUM"))
        etp = ctx.enter_context(tc.tile_pool(name="etp", bufs=2, space="PSUM"))
        otp = ctx.enter_context(tc.tile_pool(name="otp", bufs=1, space="PSUM"))
        op_ = ctx.enter_context(tc.tile_pool(name="op", bufs=1, space="PSUM"))

        ident_f8 = singles.tile([128, 128], FP8)
        make_identity(nc, ident_f8[:])
        ident_bf = singles.tile([128, 128], BF16)
        make_identity(nc, ident_bf[:])
        ident_f32 = singles.tile([128, 128], F32)
        make_identity(nc, ident_f32[:])

        # mask -> additive fp8 {-240, 0}, kept resident for all HP heads
        mneg = singles.tile([128, S // 128, S], FP8)
        for i in range(S // 128):
            mrow = mprep.tile([128, S], I32)
            nc.sync.dma_start(mrow[:], mask_d[i * 128:(i + 1) * 128, :])
            nc.scalar.activation(mneg[:, i, :], mrow[:], AF.Copy,
                                 bias=-MASK_NEG, scale=MASK_NEG)

        for h in range(HP):
            qt_raw = loadp.tile([D, S], F32, tag="qk_raw")
            kt_raw = loadp.tile([D, S], F32, tag="qk_raw")
            v_raw = loadp.tile([128, n_kc, D], F32, tag="v_raw")
            nc.sync.dma_start(qt_raw[:], qT_d[h])
            nc.sync.dma_start(kt_raw[:], kT_d[h])
            nc.sync.dma_start(
                v_raw[:], v_d[h].rearrange("(c p) d -> p c d", p=128))
            qtr = roundp.tile([D, S], F32R, tag="qtr")
            ktr = roundp.tile([D, S], F32R, tag="ktr")
            vbf = roundp.tile([128, n_kc, D], BF16, tag="vbf")
            nc.vector.tensor_copy(qtr[:], qt_raw[:])
            nc.vector.tensor_copy(ktr[:], kt_raw[:])
            nc.vector.tensor_copy(vbf[:], v_raw[:])

            for qg in range(n_qg):
                stats = statp.tile([128, n_qt, 4], F32)
                es = []
                for qt in range(n_qt):
                    q0 = qg * QG + qt * QT
                    qrow = q0 // 128
                    e = ep.tile([128, S], F32, tag="e")
                    for kvb in range(n_kvb):
                        ps = psp.tile([128, KVB], F32, tag="s")
                        for nb in range(n_nb_kvb):
                            kv0 = kvb * KVB + nb * NB
                            sl = slice(nb * NB, (nb + 1) * NB)
                            nc.tensor.matmul(
                                ps[:, sl], ident_f8[:],
                                mneg[:, qrow, kv0:kv0 + NB],
                                start=True, stop=False)
                            nc.tensor.matmul(
                                ps[:, sl], qtr[:, q0:q0 + QT],
                                ktr[:, kv0:kv0 + NB],
                                start=False, stop=True)
                        nc.scalar.activation(
                            e[:, kvb * KVB:(kvb + 1) * KVB], ps[:], AF.Exp,
                            scale=0.125, accum_out=stats[:, qt, kvb:kvb + 1])
                    # row sum of partials -> recip in stats[:, qt, 3]
                    if n_kvb == 1:
                        nc.vector.reciprocal(stats[:, qt, 3:4],
                                             stats[:, qt, 0:1])
                    else:
                        nc.vector.tensor_add(stats[:, qt, 2:3],
                                             stats[:, qt, 0:1],
                                             stats[:, qt, 1:2])
                        for kvb in range(2, n_kvb):
                            nc.vector.tensor_add(stats[:, qt, 2:3],
                                                 stats[:, qt, 2:3],
                                                 stats[:, qt, kvb:kvb + 1])
                        nc.vector.reciprocal(stats[:, qt, 3:4],
                                             stats[:, qt, 2:3])
                    es.append(e)

                # bf16 copies for the transpose path
                ebfs = []
                for qt in range(n_qt):
                    ebf = ebfp.tile([128, S], BF16, tag="ebf")
                    nc.vector.tensor_copy(ebf[:], es[qt][:])
                    ebfs.append(ebf)

                # oT[d, qg] = sum_c v[c].T @ eT[c]
                ot_ps = otp.tile([D, QG], F32)
                for c in range(n_kc):
                    et_ps = etp.tile([128, QG], BF16)
                    for j in range(n_qt):
                        nc.tensor.matmul(
                            et_ps[:, j * QT:(j + 1) * QT],
                            ebfs[j][:, c * 128:(c + 1) * 128], ident_bf[:],
                            is_transpose=True,
                            start=(j == 0), stop=(j == n_qt - 1))
                    et_s = etsp.tile([128, QG], BF16, tag="ets")
                    nc.vector.tensor_copy(et_s[:], et_ps[:])
                    nc.tensor.matmul(ot_ps[:], vbf[:, c, :], et_s[:],
                                     start=(c == 0), stop=(c == n_kc - 1))

                # p_attn rows: normalize in place and stream out
                for qt in range(n_qt):
                    q0 = qg * QG + qt * QT
                    nc.vector.tensor_scalar_mul(es[qt][:], es[qt][:],
                                                stats[:, qt, 3:4])
                    nc.sync.dma_start(p_d[h, q0:q0 + QT, :], es[qt][:])

                # un-transpose oT -> [q, d], scale by recip, stream out
                ot_s = smallp.tile([D, QG], F32, tag="ot_s")
                nc.vector.tensor_copy(ot_s[:], ot_ps[:])
                o_ps = op_.tile([128, n_qt, D], F32)
                for j in range(n_qt):
                    nc.tensor.matmul(
                        o_ps[:, j, :], ot_s[:, j * QT:(j + 1) * QT],
                        ident_f32[:D, :D], is_transpose=True,
                        start=(j == 0), stop=(j == n_qt - 1))
                o_s = smallp.tile([128, n_qt, D], F32, tag="o_s")
                for j in range(n_qt):
                    nc.vector.tensor_scalar_mul(o_s[:, j, :], o_ps[:, j, :],
                                                stats[:, j, 3:4])
                nc.sync.dma_start(
                    o_d[h, qg * QG:(qg + 1) * QG, :].rearrange(
                        "(j p) d -> p j d", p=128),
                    o_s[:])


_NC_CACHE = {}


def _get_nc(HP, S, D):
    key = (HP, S, D)
    if key not in _NC_CACHE:
        _NC_CACHE[key] = build_nc(HP=HP, S=S, D=D)
    return _NC_CACHE[key]


def kernel(query, key, value, mask):
    B, H, S, D = query.shape
    n_cores = 8
    hp = B * H // n_cores  # heads per core
    hpb = H // hp          # cores per batch

    qT = query.transpose(0, 1, 3, 2)
    kT = key.transpose(0, 1, 3, 2)

    in_maps = []
    for c in range(n_cores):
        b, h0 = c // hpb, (c % hpb) * hp
        in_maps.append({
            "qT": np.ascontiguousarray(qT[b, h0:h0 + hp]),
            "kT": np.ascontiguousarray(kT[b, h0:h0 + hp]),
            "v": np.ascontiguousarray(value[b, h0:h0 + hp]),
            "mask": np.ascontiguousarray(mask[b, 0]),
        })

    nc = _get_nc(hp, S, D)
    res = run_bass_kernel_spmd(nc, in_maps, core_ids=list(range(n_cores)))

    out = np.empty((B, H, S, D), np.float32)
    p = np.empty((B, H, S, S), np.float32)
    for c in range(n_cores):
        b, h0 = c // hpb, (c % hpb) * hp
        out[b, h0:h0 + hp] = res.results[c]["o"]
        p[b, h0:h0 + hp] = res.results[c]["p"]
    return out, p
